# revision 11
# baseline (speedup 1.0000x reference)
"""Cross-attention Trainium2 kernel (Bass/Tile), data-parallel over batch on 8 cores.

Weight-folded formulation (exact algebra, per batch element b):
    G  = Wq @ Wk^T      [D, D]   (host, fp64 accumulate -> fp32)
    W2 = Wv @ Wo        [D, D]   (host)
    H      = S2[b] @ G            [N2, D]
    scores = H @ S1[b]^T          [N2, N1]   (== (S2 Wq)(S1 Wk)^T, no 1/sqrt(d))
    A      = softmax(scores, -1)  (no max subtraction; |score| <= ~70, exp in fp32 range)
    out[b] = (A @ S1[b]) @ W2 + bo

This cuts device matmul work from 12.9 GMAC/core to 5.4 GMAC/core vs computing
Q/K/V explicitly; the folded [D, D] weight products are input-independent.

Device layout (feature dims on SBUF partitions, per 512-query chunk):
    HT = G^T @ S2T chunk          [d, n]   contraction-outer so the first
         matmuls only need the first DMA slices (fast start)
    scoresT tiles [m-part, n]     (stat s1t[d, m-tile] bf16, mov HT f32r)
    exp -> e tiles bf16 (scalar engine); ones-matmul row sums in PSUM, interleaved
    UT[d, n] = sum_m S1[m, d] e[m, n]  (stat s1r bf16, mov e)  4 PSUM banks
    UT evicted UNnormalized (scalar-engine copies); outT = W2^T @ UT;
    1/rowsum via approx reciprocal, broadcast across partitions with a tiny
    rank-1 PE matmul (ones[1,128] x recip[1,512]); normalization + bias on
    outT eviction so the chain hides under the next chunk's HT matmuls.

Scores-path operands stay f32r (bf16 keys fail the 2e-2 gate: exp amplifies
score error at |score|~60; measured 4.5e-2). The e/UT/outT path is bf16.
All device input tensors are host-rearranged chunk-major and partition-
contiguous so each load is one cheap HWDGE DIRECT2D; the two HWDGE queues
(sync + scalar) split the critical startup loads.
"""
import sys

sys.path.insert(0, "/opt/trn_rl_repo")

import numpy as np
from contextlib import ExitStack

P = 128
N_CORES = 8
B = 8          # batch (one element per core)
NQ = 2048      # queries (N2)
NK = 2048      # keys (N1)
D = 512        # query/cross dim
CHUNK = 512    # query-chunk width (moving free dim; one PSUM bank at f32)
DT = D // P    # 4

_cache = {}


def _build(nq=NQ, nk=NK):
    import concourse.tile as tile
    from concourse import bacc, mybir

    F32 = mybir.dt.float32
    F32R = mybir.dt.float32r
    BF16 = mybir.dt.bfloat16
    Exp = mybir.ActivationFunctionType.Exp
    Copy = mybir.ActivationFunctionType.Copy

    n_chunks = nq // CHUNK   # 4
    m_tiles = nk // P        # 16
    m_chunks = nk // CHUNK   # 4

    nc = bacc.Bacc("TRN2", target_bir_lowering=False, debug=False)

    # host-rearranged inputs, partition-contiguous
    S1TH = nc.dram_tensor("S1TH", [m_chunks, P, DT, CHUNK], F32R,
                          kind="ExternalInput").ap()
    S1RH = nc.dram_tensor("S1RH", [m_chunks, P, DT, D], BF16,
                          kind="ExternalInput").ap()
    S2H = nc.dram_tensor("S2H", [n_chunks, P, DT, CHUNK], F32R,
                         kind="ExternalInput").ap()
    GH = nc.dram_tensor("GH", [P, DT, D], F32R, kind="ExternalInput").ap()
    W2H = nc.dram_tensor("W2H", [P, DT, D], BF16, kind="ExternalInput").ap()
    BO = nc.dram_tensor("BO", [P, DT], F32, kind="ExternalInput").ap()
    OUT = nc.dram_tensor("OUT", [D, nq], F32, kind="ExternalOutput").ap()

    with tile.TileContext(nc) as tc, ExitStack() as ctx, \
            nc.allow_low_precision(reason="float32r/bf16 staging for matmul operands"):
        const = ctx.enter_context(tc.tile_pool(name="const", bufs=1))
        w_pool = ctx.enter_context(tc.tile_pool(name="w_pool", bufs=1))
        s2_pool = ctx.enter_context(tc.tile_pool(name="s2_pool", bufs=2))
        ht_pool = ctx.enter_context(tc.tile_pool(name="ht_pool", bufs=2))
        e_pool = ctx.enter_context(tc.tile_pool(name="e_pool", bufs=m_tiles + 2))
        mk_pool = ctx.enter_context(tc.tile_pool(name="mk_pool", bufs=6))
        out_pool = ctx.enter_context(tc.tile_pool(name="out_pool", bufs=4))
        misc = ctx.enter_context(tc.tile_pool(name="misc", bufs=2))
        ps_s = ctx.enter_context(tc.tile_pool(name="ps_s", bufs=3, space="PSUM"))
        ps_ut = ctx.enter_context(tc.tile_pool(name="ps_ut", bufs=4, space="PSUM"))
        ps_sum = ctx.enter_context(tc.tile_pool(name="ps_sum", bufs=1, space="PSUM"))

        # constants (gpsimd queue: keeps both HWDGE queues clear)
        ones_col = const.tile([P, 1], BF16, name="ones_col")
        nc.any.memset(ones_col[:], 1.0)
        ones_row_f = const.tile([1, P], F32, name="ones_row_f")
        nc.any.memset(ones_row_f[:], 1.0)
        ones_row = const.tile([1, P], F32R, name="ones_row")
        nc.vector.tensor_copy(ones_row[:], ones_row_f[:])
        bo_t = const.tile([P, DT], F32, name="bo_t")
        nc.gpsimd.dma_start(bo_t[:], BO[:, :])

        # persistent operands, one tile per DMA slice so dependency tracking
        # is exactly per-transfer (first compute starts on the first slices)
        g_list = [w_pool.tile([P, D], F32R, name=f"g_{it}") for it in range(DT)]
        s1t_list = [w_pool.tile([P, DT, CHUNK], F32R, name=f"s1t_{mc}")
                    for mc in range(m_chunks)]
        s1r_list = [w_pool.tile([P, DT, D], BF16, name=f"s1r_{mc}")
                    for mc in range(m_chunks)]
        w2_t = w_pool.tile([P, DT, D], BF16, name="w2_t")        # W2: [d-part, dt, j]

        # startup DMAs, two HWDGE queues in parallel:
        #   scalar queue: g slices (HT stationaries) -> s1t (scores
        #   stationaries, first needed ~7us in) -> w2
        #   sync queue: s2 chunk-0 slices (HT moving) -> s1r (UT stationaries)
        s2c0 = [w_pool.tile([P, CHUNK], F32R, name=f"s2c0_{it}")
                for it in range(DT)]
        for it in range(DT):
            nc.scalar.dma_start(g_list[it][:], GH[:, it, :])
            nc.sync.dma_start(s2c0[it][:], S2H[0, :, it, :])
        for mc in range(m_chunks):
            nc.sync.dma_start(s1t_list[mc][:], S1TH[mc, :, :, :])
            nc.scalar.dma_start(s1r_list[mc][:], S1RH[mc, :, :, :])
        nc.scalar.dma_start(w2_t[:], W2H[:, :, :])

        def load_s2(c):
            t = s2_pool.tile([P, DT, CHUNK], F32R, name="s2_t", tag="s2")
            nc.sync.dma_start(t[:], S2H[c, :, :, :])
            return t

        def emit_ht(s2_ap):
            # contraction(it)-outer accumulation: 4 concurrent PSUM tiles
            # (borrows ps_ut, which is idle between UT eviction and next use)
            accs = [ps_ut.tile([P, CHUNK], F32, name="acc_h", tag="ut")
                    for _ in range(DT)]
            for it in range(DT):
                for jt in range(DT):
                    nc.tensor.matmul(
                        accs[jt][:], g_list[it][:, jt * P:(jt + 1) * P],
                        s2_ap(it),
                        start=(it == 0), stop=(it == DT - 1))
            ht = ht_pool.tile([P, DT, CHUNK], F32R, name="ht_t", tag="ht")
            for jt in range(DT):
                nc.vector.tensor_copy(ht[:, jt, :], accs[jt][:])
            return ht

        def emit_ut(ut_list, e_t, mt):
            for dt_ in range(DT):
                nc.tensor.matmul(
                    ut_list[dt_][:],
                    s1r_list[mt // DT][:, mt % DT, dt_ * P:(dt_ + 1) * P], e_t[:],
                    start=(mt == 0), stop=(mt == m_tiles - 1))

        ht_cur = emit_ht(lambda it: s2c0[it][:])
        for c in range(n_chunks):
          with nc.named_scope(f"chunk{c}"):
            csl = slice(c * CHUNK, (c + 1) * CHUNK)
            if c + 1 < n_chunks:
                s2_nxt_t = load_s2(c + 1)
                s2_nxt = (lambda t: (lambda it: t[:, it, :]))(s2_nxt_t)

            # scoresT tiles + exp, with UT and rowsum for mt-1 pipelined
            # behind so the scalar-engine exp latency hides under the next
            # scores matmuls
            sum_ps = ps_sum.tile([1, CHUNK], F32, name="sum_ps", tag="sum")
            ut_list = [
                ps_ut.tile([P, CHUNK], F32, name="ut", tag="ut")
                for _ in range(DT)
            ]
            e_list = []
            for mt in range(m_tiles):
                acc_s = ps_s.tile([P, CHUNK], F32, name="acc_s", tag="ps")
                for dt_ in range(DT):
                    nc.tensor.matmul(
                        acc_s[:],
                        s1t_list[mt // DT][:, dt_, (mt % DT) * P:(mt % DT + 1) * P],
                        ht_cur[:, dt_, :],
                        start=(dt_ == 0), stop=(dt_ == DT - 1))
                e_t = e_pool.tile([P, CHUNK], BF16, name="e_t", tag="e")
                nc.scalar.activation(e_t[:], acc_s[:], Exp)
                e_list.append(e_t)
                if mt > 0:
                    emit_ut(ut_list, e_list[mt - 1], mt - 1)
                    nc.tensor.matmul(
                        sum_ps[:], ones_col[:], e_list[mt - 1][:],
                        start=(mt - 1 == 0), stop=False)
            emit_ut(ut_list, e_list[m_tiles - 1], m_tiles - 1)
            nc.tensor.matmul(
                sum_ps[:], ones_col[:], e_list[m_tiles - 1][:],
                start=False, stop=True)

            last = (c + 1 == n_chunks)

            def evict_ut():
                # unnormalized UT eviction split across scalar + vector so
                # m_list is ready ~2 copies after UT(15) on either engine
                m_list = []
                for dt_ in range(DT):
                    m_t = mk_pool.tile([P, CHUNK], BF16, name="m_t", tag="mk")
                    if dt_ % 2 == 0:
                        nc.scalar.activation(m_t[:], ut_list[dt_][:], Copy)
                    else:
                        nc.vector.tensor_copy(m_t[:], ut_list[dt_][:])
                    m_list.append(m_t)
                return m_list

            def recip_chain():
                # 1/rowsum (vector approx reciprocal at ~22 bits)
                sum_sb = misc.tile([1, CHUNK], F32, name="sum_sb", tag="ssb")
                nc.vector.tensor_copy(sum_sb[:], sum_ps[:])
                recip = misc.tile([1, CHUNK], F32, name="recip", tag="rec")
                rscr = misc.tile([1, CHUNK], F32, name="rscr", tag="rscr")
                nc.vector.reciprocal_approx_accurate(recip[:], sum_sb[:], rscr[:])
                recip_r = misc.tile([1, CHUNK], F32R, name="recip_r", tag="recr")
                nc.vector.tensor_copy(recip_r[:], recip[:])
                return recip_r

            def emit_bc(recip_r):
                # broadcast 1/rowsum to all partitions: rank-1 PE matmul
                bc_ps = ps_s.tile([P, CHUNK], F32, name="bc_ps", tag="ps")
                nc.tensor.matmul(bc_ps[:], ones_row[:], recip_r[:],
                                 start=True, stop=True)
                bc = misc.tile([P, CHUNK], F32, name="bc", tag="bc")
                nc.vector.tensor_copy(bc[:], bc_ps[:])
                return bc

            def emit_out_mms(jt, m_list):
                acc_o = ps_s.tile([P, CHUNK], F32, name="acc_o", tag="ps")
                for dt_ in range(DT):
                    nc.tensor.matmul(
                        acc_o[:], w2_t[:, dt_, jt * P:(jt + 1) * P],
                        m_list[dt_][:],
                        start=(dt_ == 0), stop=(dt_ == DT - 1))
                return acc_o

            def evict_out(jt, acc_o, bc):
                # normalize on vector (gpsimd cannot read PSUM); bias-add
                # alternates vector / gpsimd(Pool) to split the drain
                o_nm = out_pool.tile([P, CHUNK], F32, name="o_nm", tag="osb")
                nc.vector.tensor_mul(o_nm[:], acc_o[:], bc[:])
                eng = nc.vector if jt % 2 == 0 else nc.gpsimd
                o_sb = out_pool.tile([P, CHUNK], F32, name="o_sb", tag="osb")
                eng.tensor_scalar_add(o_sb[:], o_nm[:], bo_t[:, jt:jt + 1])
                nc.sync.dma_start(OUT[jt * P:(jt + 1) * P, csl], o_sb[:])

            if not last:
                recip_r = recip_chain()
                m_list = evict_ut()
                ht_cur = emit_ht(s2_nxt)
                bc = emit_bc(recip_r)  # after HT: reciprocal already done
                for jt in range(DT):
                    acc_o = emit_out_mms(jt, m_list)
                    evict_out(jt, acc_o, bc)
            else:
                # no next-chunk HT to hide behind: evict UT first (two
                # engines), run outT unnormalized in pairs (ps_s bufs=3:
                # two acc_o tiles + bc_ps), broadcast between the pairs
                m_list = evict_ut()
                recip_r = recip_chain()
                acc0 = emit_out_mms(0, m_list)
                acc1 = emit_out_mms(1, m_list)
                bc = emit_bc(recip_r)
                evict_out(0, acc0, bc)
                evict_out(1, acc1, bc)
                acc2 = emit_out_mms(2, m_list)
                acc3 = emit_out_mms(3, m_list)
                evict_out(2, acc2, bc)
                evict_out(3, acc3, bc)

    nc.compile()
    return nc


def _get_nc(nq=NQ, nk=NK):
    key = (nq, nk)
    if key not in _cache:
        _cache[key] = _build(nq, nk)
    return _cache[key]


def kernel(S1, S2, Wq, Wk, Wv, Wo, bo, _trace=False):
    from concourse.bass_utils import run_bass_kernel_spmd
    import ml_dtypes

    S1 = np.asarray(S1, np.float32)
    S2 = np.asarray(S2, np.float32)
    b, nk, _ = S1.shape
    _, nq, _ = S2.shape
    nc = _get_nc(nq, nk)

    # fold the weight pairs (input-independent): G = Wq Wk^T, W2 = Wv Wo
    Wq64 = np.asarray(Wq, np.float64)
    Wk64 = np.asarray(Wk, np.float64)
    Wv64 = np.asarray(Wv, np.float64)
    Wo64 = np.asarray(Wo, np.float64)
    G = (Wq64 @ Wk64.T).astype(np.float32)
    W2 = (Wv64 @ Wo64).astype(np.float32)
    bo_r = np.ascontiguousarray(
        np.asarray(bo, np.float32).reshape(DT, P).T)  # [128, DT]

    # partition-contiguous device layouts
    # [n, d] -> [n_chunks, 128(p), DT, chunk]: x[c, p, t, j] = a[c*CH+j, t*128+p]
    def chunkmaj(a, dtype=np.float32):
        nch = a.shape[0] // CHUNK
        x = a.reshape(nch, CHUNK, DT, P).transpose(0, 3, 2, 1)
        return np.ascontiguousarray(x.astype(dtype))

    # [m, d] -> [m_chunks, 128(p), DT(mtl), D]: x[mc, p, l, d] = a[mc*CH+l*128+p, d]
    def rowmaj(a, dtype):
        mch = a.shape[0] // CHUNK
        x = a.reshape(mch, DT, P, D).transpose(0, 2, 1, 3)
        return np.ascontiguousarray(x.astype(dtype))

    # [d_in, d_out] -> [128(p), DT, D]: x[p, t, j] = w[t*128+p, j]
    def wmaj(w, dtype):
        x = w.reshape(DT, P, D).transpose(1, 0, 2)
        return np.ascontiguousarray(x.astype(dtype))

    GH = wmaj(G, np.float32)
    W2H = wmaj(W2, ml_dtypes.bfloat16)

    in_maps = []
    for i in range(b):
        in_maps.append({
            "S1TH": chunkmaj(S1[i]),                      # scores stationaries
            "S1RH": rowmaj(S1[i], ml_dtypes.bfloat16),    # UT stationaries
            "S2H": chunkmaj(S2[i]),
            "GH": GH, "W2H": W2H, "BO": bo_r,
        })

    res = run_bass_kernel_spmd(nc, in_maps, list(range(b)), trace=_trace)
    out = np.stack([np.asarray(res.results[i]["OUT"]).T for i in range(b)])
    if _trace:
        kernel.last_result = res
    return np.ascontiguousarray(out.astype(np.float32))
